# revision 65
# baseline (speedup 1.0000x reference)
"""Cross-attention 1d kernel for Trainium2 (Bass/Tile), SPMD over 8 NeuronCores.

Problem (hardcoded shapes): N=4, C=512, L=2048, H=8, D=64.
  out_a = out_a_w @ attn(a_norm -> b_norm) + out_a_b + a
  out_b = out_b_w @ attn(b_norm -> a_norm) + out_b_b + b

Sharding: 8 cores = 4 samples x 2 directions (a->b, b->a). Each core computes
one full [512, 2048] output tensor. No cross-core communication.

v3 design notes (vs v2):
  - ACT+DVE are the bottleneck: every PSUM->SBUF crossing must use them
    (gpsimd is BIR-forbidden from PSUM), and the softmax exp alone is 262144
    128-row columns.  All changes cut their load or idle time:
    * GroupNorm computed on the HOST (exact mean/var of the full input) and
      folded into the projections: weights are premultiplied by
      gn_w * rstd * WS, biases become host constants (vecs), and the
      fp8 projection inputs x8/y8 are cast on the host too.  No device
      stats, no device casts; the bf16 residual source x loads last.
    * all input DMAs issue from the SP HWDGE queue (a dma_start on the
      ACT/DVE queues blocks that sequencer ~1.5us each).
    * out-projection copy+bias+residual fused: either one DVE
      scalar_tensor_tensor (os = psum*OUT_SC + x_sb, bo pre-folded into
      x_sb by gpsimd) or an ACT copy plus a gpsimd residual add; output
      stores are bf16 and batched per q-block.
    * exp split ACT (accurate exp, fp8 out) / DVE (Schraudolph i8 bitcast)
      via a greedy balance with measured cost constants
      (ACT n*0.8333+185, DVE n*1.0417+125), reset at conveyor start.
      Softmax tails (reciprocal+mult; the BIR verifier rejects a fused
      tensor_tensor divide) stay on DVE and are counted in the balance.
  - big constant regions (q8 zero slot, vaug ones columns) are memset on
    Pool/DVE under the input DMA, split in needed-by order.
  - same conveyor skeleton as v2: 3-deep PSUM ring [128,2,512] for
    projections/scores/out, double-buffered [128,512] av accumulators,
    av lags 4 windows behind scores so the in-order PE never blocks the
    exp engines; the final q-block's out pieces run 512-wide on DVE with
    per-half stores to shorten the drain.
"""

import sys

sys.path.insert(0, "/opt/trn_rl_repo")

import numpy as np
import ml_dtypes

import concourse.bass as bass
import concourse.tile as tile
from concourse import bacc, mybir
from concourse.bass import ts
from concourse.bass_utils import run_bass_kernel_spmd

F32 = mybir.dt.float32
BF16 = mybir.dt.bfloat16
FP8 = mybir.dt.float8e4
I8 = mybir.dt.int8
AF = mybir.ActivationFunctionType
ALU = mybir.AluOpType
DR = mybir.MatmulPerfMode.DoubleRow
E4 = ml_dtypes.float8_e4m3
BF16_NP = ml_dtypes.bfloat16

N, C, L, H = 4, 512, 2048, 8
D, P = 64, 128
CO = C // P          # 4 channel chunks
LT = L // P          # 16 k-position tiles
QQ = 4               # 512-wide query chunks
QW = L // QQ
EPS = 1e-5
SCALE = float(D) ** -0.5

WS = 32.0            # host-side weight prescale (wq/wk/wv/wo)
QS = 32.0            # q fp8 scale (on top of SCALE)
KS = 4.0             # k fp8 scale
VS = 4.0             # v fp8 scale
AS = 64.0            # attn fp8 scale
ONEC = VS / AS       # ones-column value -> denominator lands pre-scaled
EXPS = 1.0 / (QS * KS)
K_SCH = 8.0 / np.log(2.0) * EXPS
B_SCH = 55.55        # calibrated against the real (round-to-nearest) path
OUT_SC = 1.0 / (WS * AS)


def _build_module():
    nc = bacc.Bacc("TRN2", target_bir_lowering=False, debug=False, num_devices=8)

    def din(name, shape, dt=F32):
        return nc.dram_tensor(name, list(shape), dt, kind="ExternalInput")

    x_d = din("x", (C, L), BF16)      # query-side input (residual only)
    x8_d = din("x8", (C, L), FP8)     # host-cast fp8 of x (projection input)
    y8_d = din("y8", (C, L), FP8)     # host-cast fp8 of y (k/v input)
    wq8_d = din("wq8", (C, C), FP8)   # ((w*gn_w*rstd).T * WS) fp8 [c_in,c_out]
    wk8_d = din("wk8", (C, C), FP8)
    wv8_d = din("wv8", (C, C), FP8)
    wo8_d = din("wo8", (C, C), FP8)
    # bq_eff*SCALE*QS, bk_eff*KS, bo_eff  (host-folded, see _core_inputs)
    vecs_d = din("vecs", (3 * C,))
    out_d = nc.dram_tensor("out", [C, L], BF16, kind="ExternalOutput")

    # build-time engine-load estimates (ns) for the greedy ACT/DVE split
    est = {"A": 0.0, "D": 0.0}

    def cost(eng, units):
        if eng == "A":
            return units * 0.8333 + 185.0
        return units * 1.0417 + 125.0

    def pick():
        return "A" if est["A"] <= est["D"] else "D"

    with tile.TileContext(nc) as tc:
        with (
            tc.tile_pool(name="persist", bufs=1) as pp,
            tc.tile_pool(name="small", bufs=1) as sp,
        ):
            x_sb = pp.tile([P, CO, L], BF16)     # 16K/part (residual source)
            x8 = pp.tile([P, CO, L], FP8)        # 8K  host-cast fp8 of x
            y8 = pp.tile([P, CO, L], FP8)        # 8K
            q8 = pp.tile([P, CO, 2, L], FP8)     # 16K (slot 1 = zeros)
            k8 = pp.tile([P, CO, L], FP8)        # 8K
            vaug = pp.tile([P, LT, H, P], FP8)   # 16K (cols 64:128 = ONEC)
            attn8 = pp.tile([P, CO, L], FP8)     # 8K
            wq8 = pp.tile([P, CO, C], FP8)       # 2K each
            wk8 = pp.tile([P, CO, C], FP8)
            wv8 = pp.tile([P, CO, C], FP8)
            wo8 = pp.tile([P, CO, C], FP8)

            vecs_pc = sp.tile([P, 3, CO], F32)
            bq_pc = vecs_pc[:, 0, :]
            bk_pc = vecs_pc[:, 1, :]
            bo_pc = vecs_pc[:, 2, :]

            # constant regions, under the input DMA (no data deps); DVE takes
            # only the late-needed vaug half (it must be free for k-copies by
            # ~6.5us), Pool interleaves the rest in needed-by order
            nc.vector.memset(vaug[:, LT // 2:LT, :, D:P], ONEC)
            nc.gpsimd.memset(vaug[:, 0:LT // 4, :, D:P], ONEC)
            nc.gpsimd.memset(q8[:, 0, 1, :], 0.0)
            nc.gpsimd.memset(q8[:, 1, 1, :], 0.0)
            nc.gpsimd.memset(vaug[:, LT // 4:LT // 2, :, D:P], ONEC)
            nc.gpsimd.memset(q8[:, 2, 1, :], 0.0)
            nc.gpsimd.memset(q8[:, 3, 1, :], 0.0)

            # ---- input DMAs, all on the SP HWDGE queue so the ACT/DVE
            # sequencers never block on DMA issue; fp8 projection inputs and
            # weights first (k-side before q-side), the bf16 residual source
            # last (first needed ~35us in)
            nc.sync.dma_start(
                vecs_pc[:], vecs_d[:].rearrange("(p t co) -> p t co", p=P, t=3))
            LH = L // 2
            y8_r = y8_d[:].rearrange("(co p) l -> p co l", p=P)
            x8_r = x8_d[:].rearrange("(co p) l -> p co l", p=P)
            # column-halved input loads: each projection contracts all
            # channels but only for its own columns, so the first half of
            # k/q-proj can start as soon as its half-tensor lands
            nc.sync.dma_start(wk8[:], wk8_d[:].rearrange("(ko p) o -> p ko o", p=P))
            nc.sync.dma_start(y8[:, :, 0:LH], y8_r[:, :, 0:LH])
            nc.sync.dma_start(wv8[:], wv8_d[:].rearrange("(ko p) o -> p ko o", p=P))
            nc.sync.dma_start(y8[:, :, LH:L], y8_r[:, :, LH:L])
            nc.sync.dma_start(x8[:, :, 0:LH], x8_r[:, :, 0:LH])
            nc.sync.dma_start(wq8[:], wq8_d[:].rearrange("(ko p) o -> p ko o", p=P))
            nc.sync.dma_start(x8[:, :, LH:L], x8_r[:, :, LH:L])
            nc.sync.dma_start(wo8[:], wo8_d[:].rearrange("(ko p) o -> p ko o", p=P))
            nc.sync.dma_start(x_sb[:], x_d[:].rearrange("(co p) l -> p co l", p=P))

            # fold output bias into the residual source (gpsimd, off stream)
            for mo in range(CO):
                nc.gpsimd.tensor_scalar(x_sb[:, mo, :], x_sb[:, mo, :],
                                        bo_pc[:, mo:mo + 1], 0.0,
                                        op0=ALU.add, op1=ALU.add)

            # ================= conveyor: proj -> attention -> out-proj ===
            # DVE's vaug memset is only partly hidden by its idle lead-in
            est["A"] = 0.0
            est["D"] = 2000.0
            with (
                tc.tile_pool(name="ring", bufs=3, space="PSUM") as rsp,
                tc.tile_pool(name="oh", bufs=2, space="PSUM") as ohp,
                tc.tile_pool(name="ptp", bufs=10) as ptp,
                tc.tile_pool(name="rpool", bufs=3) as rp,
                tc.tile_pool(name="opool", bufs=3) as op_,
                tc.tile_pool(name="ospool", bufs=3) as osp,
            ):
                def take2():
                    rt = rsp.tile([P, 2, QW], F32, tag="ring")
                    return rt

                def psum_copy_scale_bias(dst, src, scale_imm, bias_ap, units):
                    """dst = src*scale + bias via ACT or DVE (greedy)."""
                    eng = pick()
                    est[eng] += cost(eng, units)
                    if eng == "A":
                        nc.scalar.activation(dst, src, AF.Identity,
                                             bias=bias_ap, scale=scale_imm)
                    else:
                        nc.vector.tensor_scalar(dst, src, scale_imm, bias_ap,
                                                op0=ALU.mult, op1=ALU.add)

                def psum_copy_scale(dst, src, scale_imm, units):
                    eng = pick()
                    est[eng] += cost(eng, units)
                    if eng == "A":
                        nc.scalar.mul(dst, src, scale_imm)
                    else:
                        nc.vector.tensor_scalar(dst, src, scale_imm, 0.0,
                                                op0=ALU.mult, op1=ALU.add)

                def emit_kq(side, p, lc2):
                    rt = take2()
                    w8 = wk8 if side == "k" else wq8
                    src = y8 if side == "k" else x8
                    for j in range(2):
                        lc = 2 * lc2 + j
                        for m in range(2):
                            nc.tensor.matmul(
                                rt[:, j, :],
                                w8[:, 2 * m:2 * m + 2, ts(p, P)],
                                src[:, 2 * m:2 * m + 2, ts(lc, QW)],
                                start=(m == 0), stop=(m == 1), perf_mode=DR)
                    if side == "k":
                        dst = k8[:, p, 2 * lc2 * QW:(2 * lc2 + 2) * QW]
                        dst = dst.rearrange("p (a b) -> p a b", a=2)
                        psum_copy_scale_bias(dst, rt[:], KS / WS,
                                             bk_pc[:, p:p + 1], 1024)
                    else:
                        dst = q8[:, p, 0, 2 * lc2 * QW:(2 * lc2 + 2) * QW]
                        dst = dst.rearrange("p (a b) -> p a b", a=2)
                        psum_copy_scale_bias(dst, rt[:], SCALE * QS / WS,
                                             bq_pc[:, p:p + 1], 1024)

                def emit_vp(lt2):
                    rt = take2()
                    for i in range(2):
                        lt = 2 * lt2 + i
                        for m in range(2):
                            nc.tensor.matmul(
                                rt[:, i, :],
                                y8[:, 2 * m:2 * m + 2, ts(lt, P)],
                                wv8[:, 2 * m:2 * m + 2, :],
                                start=(m == 0), stop=(m == 1), perf_mode=DR)
                    dst = vaug[:, 2 * lt2:2 * lt2 + 2, :, 0:D]
                    src = rt[:].rearrange("p a (h d) -> p a h d", d=D)
                    psum_copy_scale(dst, src, VS / WS, 1024)

                oh_cur = {}

                def emit_attn_scores(qq, p, h, kt2):
                    rt = take2()
                    lo = D * h
                    qs = qq * QW
                    for j in range(2):
                        kt = 2 * kt2 + j
                        lhsT = (k8[lo:lo + D, p, ts(kt, P)]
                                .unsqueeze(1).broadcast_to([D, 2, P]))
                        nc.tensor.matmul(rt[:, j, :], lhsT,
                                         q8[lo:lo + D, p, :, qs:qs + QW],
                                         start=True, stop=True, perf_mode=DR)
                    return rt

                def emit_exp(rt):
                    pt_t = ptp.tile([P, 2, QW], FP8, tag="pt")
                    eng = pick()
                    est[eng] += cost(eng, 2 * QW)
                    if eng == "A":
                        nc.scalar.activation(pt_t[:], rt[:],
                                             AF.Exp, bias=0.0, scale=EXPS)
                    else:
                        nc.vector.tensor_scalar(
                            pt_t[:].bitcast(I8), rt[:], K_SCH, B_SCH,
                            op0=ALU.mult, op1=ALU.add)
                    return pt_t

                pend_tails = []

                def emit_attn_av(qq, p, h, kt2, pt_t):
                    if kt2 == 0:
                        oh_t = ohp.tile([P, QW], F32, tag="oh")
                        oh_cur[h] = oh_t
                    oh = oh_cur[h]
                    nc.tensor.matmul(oh[:], vaug[:, 2 * kt2:2 * kt2 + 2, h, :],
                                     pt_t[:],
                                     start=(kt2 == 0), stop=(kt2 == 7),
                                     perf_mode=DR)
                    if kt2 == 7:
                        # defer the tail: emitting it now would park a
                        # PE-gated op at the head of DVE's in-order queue,
                        # stalling ready exp work queued behind it
                        pend_tails.append((qq, p, h, oh))

                def emit_tail(qq, p, h, oh):
                    # tail: r = 1/den ; attn8 = num * r  (DVE only; the
                    # BIR verifier rejects tensor_tensor divide)
                    qs = qq * QW
                    lo = D * h
                    r = rp.tile([D, QW], F32, tag="r")
                    nc.vector.reciprocal(r[:], oh[D:P, :])
                    nc.vector.tensor_tensor(attn8[lo:lo + D, p, qs:qs + QW],
                                            oh[0:D, :], r[:], ALU.mult)
                    est["D"] += 2 * cost("D", QW)

                def flush_tails(n=0):
                    while len(pend_tails) > n:
                        emit_tail(*pend_tails.pop(0))

                os_cur = {}

                def emit_out(qq, mo2):
                    rt = take2()
                    qs = qq * QW
                    for i in range(2):
                        mo = 2 * mo2 + i
                        for m in range(2):
                            nc.tensor.matmul(
                                rt[:, i, :],
                                wo8[:, 2 * m:2 * m + 2, ts(mo, P)],
                                attn8[:, 2 * m:2 * m + 2, qs:qs + QW],
                                start=(m == 0), stop=(m == 1), perf_mode=DR)
                    if mo2 == 0:
                        os_t = osp.tile([P, CO, QW], BF16, tag="os")
                        os_cur[qq] = os_t
                    os_t = os_cur[qq]

                    def one_copy(sl, units, force=None):
                        dst = os_t[:, 2 * mo2:2 * mo2 + 2, :][:, sl, :]
                        src = rt[:, sl, :]
                        xsl = x_sb[:, 2 * mo2:2 * mo2 + 2, qs:qs + QW][:, sl, :]
                        eng = force or pick()
                        est[eng] += cost(eng, units)
                        if eng == "D":
                            # fused copy+residual: os = psum*OUT_SC + x_sb
                            nc.vector.scalar_tensor_tensor(
                                dst, src, OUT_SC, xsl,
                                op0=ALU.mult, op1=ALU.add)
                        else:
                            # ACT copy + residual add on the idle Pool engine
                            ot = op_.tile([P, 2, QW], F32, tag="ot")
                            nc.scalar.mul(ot[:, sl, :], src, OUT_SC)
                            nc.gpsimd.tensor_tensor(dst, ot[:, sl, :], xsl,
                                                    ALU.add)

                    if qq == QQ - 1:
                        # drain: 512-col pieces; slow ACT+Pool path first,
                        # fused DVE path last; store each half as it's done
                        one_copy(slice(0, 1), 512, "D")
                        one_copy(slice(1, 2), 512, "D")
                        nc.sync.dma_start(
                            out_d[:].rearrange("(mo p) l -> p mo l", p=P)
                            [:, 2 * mo2:2 * mo2 + 2, qs:qs + QW],
                            os_t[:, 2 * mo2:2 * mo2 + 2, :])
                    else:
                        one_copy(slice(0, 2), 1024)
                        if mo2 == 1:
                            nc.sync.dma_start(
                                out_d[:].rearrange("(mo p) l -> p mo l", p=P)
                                [:, :, qs:qs + QW], os_t[:])

                def emit_out_final(qq):
                    # final q-block: run every start-pass (channels 0:256,
                    # whose tails finish early) before any stop-pass, so the
                    # in-order PE does the m0 work while the last tails run
                    qs = qq * QW
                    rts = [take2(), take2()]
                    for m in range(2):
                        for mo2 in range(2):
                            for i in range(2):
                                mo = 2 * mo2 + i
                                nc.tensor.matmul(
                                    rts[mo2][:, i, :],
                                    wo8[:, 2 * m:2 * m + 2, ts(mo, P)],
                                    attn8[:, 2 * m:2 * m + 2, qs:qs + QW],
                                    start=(m == 0), stop=(m == 1),
                                    perf_mode=DR)
                    os_t = osp.tile([P, CO, QW], BF16, tag="os")
                    for mo2 in range(2):
                        for i in range(2):
                            dst = os_t[:, 2 * mo2 + i, :].unsqueeze(1)
                            src = rts[mo2][:, i, :].unsqueeze(1)
                            xsl = x_sb[:, 2 * mo2 + i, qs:qs + QW].unsqueeze(1)
                            est["D"] += cost("D", 512)
                            nc.vector.scalar_tensor_tensor(
                                dst, src, OUT_SC, xsl,
                                op0=ALU.mult, op1=ALU.add)
                        nc.sync.dma_start(
                            out_d[:].rearrange("(mo p) l -> p mo l", p=P)
                            [:, 2 * mo2:2 * mo2 + 2, qs:qs + QW],
                            os_t[:, 2 * mo2:2 * mo2 + 2, :])

                # ---- window stream construction (k first: wk lands first) --
                stream = []
                for lc2 in range(2):
                    stream.append(("kq", "k", 0, lc2))
                stream.append(("vp", 0))
                stream.append(("vp", 1))
                for lc2 in range(2):
                    stream.append(("kq", "q", 0, lc2))
                for qq in range(QQ):
                    for p in range(CO):
                        inter = []
                        if qq == 0 and p < 3:
                            inter = [("kq", side, p + 1, l)
                                     for side in ("k", "q") for l in range(2)]
                        if qq >= 1 and p == 0:
                            inter = [("out", qq - 1, m) for m in range(2)]
                        atw = []
                        for h in range(2):
                            for kt2 in range(8):
                                if qq == 0 and p == 0 and h == 0 and kt2 >= 2:
                                    atw.append(("vp", kt2))
                                atw.append(("attn", qq, p, h, kt2))
                        # spread `inter` evenly through the attention run
                        out2 = []
                        k = 0
                        step = max(2, (len(atw) - 1) // max(1, len(inter)))
                        for i, w in enumerate(atw):
                            out2.append(w)
                            if inter and k < len(inter) and i % step == step - 1:
                                out2.append(inter[k])
                                k += 1
                        out2.extend(inter[k:])
                        stream.extend(out2)
                stream.append(("outfinal", QQ - 1))

                # ---- emission, software-pipelined (av lags scores) ----
                pend = []

                def flush(n=0):
                    while len(pend) > n:
                        emit_attn_av(*pend.pop(0))

                for w in stream:
                    if w[0] == "kq":
                        emit_kq(w[1], w[2], w[3])
                    elif w[0] == "vp":
                        emit_vp(w[1])
                    elif w[0] == "out":
                        # out-proj reads attn8 written by pending tails
                        flush()
                        flush_tails()
                        emit_out(w[1], w[2])
                    elif w[0] == "outfinal":
                        flush()
                        flush_tails()
                        emit_out_final(w[1])
                    else:
                        rt = emit_attn_scores(*w[1:])
                        pt_t = emit_exp(rt)
                        flush(4)
                        flush_tails(0)
                        pend.append((*w[1:], pt_t))
                flush()
                flush_tails()

    nc.compile()
    return nc


_NC_CACHE = None


def _get_module():
    global _NC_CACHE
    if _NC_CACHE is None:
        _NC_CACHE = _build_module()
    return _NC_CACHE


def _core_inputs(x, y, gnx_w, gnx_b, gny_w, gny_b, qw_q, qb_q, qw_kv, qb_kv,
                 ow, ob):
    f32 = lambda a: np.asarray(a, np.float32)
    x, y = f32(x), f32(y)
    wq, bq = f32(qw_q[0:C]), f32(qb_q[0:C])
    wk, bk = f32(qw_kv[C:2 * C]), f32(qb_kv[C:2 * C])
    wv, bv = f32(qw_kv[2 * C:3 * C]), f32(qb_kv[2 * C:3 * C])
    ow, ob = f32(ow), f32(ob)
    gnx_w, gnx_b = f32(gnx_w), f32(gnx_b)
    gny_w, gny_b = f32(gny_w), f32(gny_b)
    # exact GroupNorm(1,C) stats on the host, folded into weights + biases
    mu_x, rstd_x = x.mean(), 1.0 / np.sqrt(x.var() + EPS)
    mu_y, rstd_y = y.mean(), 1.0 / np.sqrt(y.var() + EPS)
    f8 = lambda w, g, rstd: np.ascontiguousarray(
        (w * (g * rstd)[None, :]).T * WS).astype(E4)
    bq_eff = bq + wq @ (gnx_b - mu_x * rstd_x * gnx_w)
    bk_eff = bk + wk @ (gny_b - mu_y * rstd_y * gny_w)
    bv_eff = bv + wv @ (gny_b - mu_y * rstd_y * gny_w)
    bo_eff = ob + ow @ bv_eff
    # partition-major layout: one contiguous 48B run per partition keeps
    # the vecs DMA at 128 descriptors instead of 1536 7ns-minimum ones
    vecs = np.ascontiguousarray(
        np.stack([bq_eff * SCALE * QS, bk_eff * KS, bo_eff])
        .reshape(3, CO, P).transpose(2, 0, 1)).ravel()
    return {
        "x": np.ascontiguousarray(x).astype(BF16_NP),
        "x8": np.ascontiguousarray(x).astype(E4),
        "y8": np.ascontiguousarray(y).astype(E4),
        "wq8": f8(wq, gnx_w, rstd_x), "wk8": f8(wk, gny_w, rstd_y),
        "wv8": f8(wv, gny_w, rstd_y),
        "wo8": np.ascontiguousarray(ow.T * WS).astype(E4),
        "vecs": vecs,
    }


def kernel(a, b, gn_a_w, gn_a_b, gn_b_w, gn_b_b,
           qkv_a_w, qkv_a_b, qkv_b_w, qkv_b_b,
           out_a_w, out_a_b, out_b_w, out_b_b):
    a = np.asarray(a); b = np.asarray(b)
    nc = _get_module()
    in_maps = []
    for s in range(N):
        # direction a->b : q from a, k/v from b, output -> out_a[s]
        in_maps.append(_core_inputs(a[s], b[s], gn_a_w, gn_a_b, gn_b_w, gn_b_b,
                                    qkv_a_w, qkv_a_b, qkv_b_w, qkv_b_b,
                                    out_a_w, out_a_b))
        # direction b->a : q from b, k/v from a, output -> out_b[s]
        in_maps.append(_core_inputs(b[s], a[s], gn_b_w, gn_b_b, gn_a_w, gn_a_b,
                                    qkv_b_w, qkv_b_b, qkv_a_w, qkv_a_b,
                                    out_b_w, out_b_b))
    res = run_bass_kernel_spmd(nc, in_maps, core_ids=list(range(2 * N)))
    out_a = np.stack([np.asarray(res.results[2 * s]["out"]).view(BF16_NP)
                      if res.results[2 * s]["out"].dtype != BF16_NP
                      else res.results[2 * s]["out"] for s in range(N)])
    out_b = np.stack([np.asarray(res.results[2 * s + 1]["out"]).view(BF16_NP)
                      if res.results[2 * s + 1]["out"].dtype != BF16_NP
                      else res.results[2 * s + 1]["out"] for s in range(N)])
    return out_a.astype(np.float32), out_b.astype(np.float32)


# revision 66
# speedup vs baseline: 1.0008x; 1.0008x over previous
"""Cross-attention 1d kernel for Trainium2 (Bass/Tile), SPMD over 8 NeuronCores.

Problem (hardcoded shapes): N=4, C=512, L=2048, H=8, D=64.
  out_a = out_a_w @ attn(a_norm -> b_norm) + out_a_b + a
  out_b = out_b_w @ attn(b_norm -> a_norm) + out_b_b + b

Sharding: 8 cores = 4 samples x 2 directions (a->b, b->a). Each core computes
one full [512, 2048] output tensor. No cross-core communication.

v3 design notes (vs v2):
  - ACT+DVE are the bottleneck: every PSUM->SBUF crossing must use them
    (gpsimd is BIR-forbidden from PSUM), and the softmax exp alone is 262144
    128-row columns.  All changes cut their load or idle time:
    * GroupNorm computed on the HOST (exact mean/var of the full input) and
      folded into the projections: weights are premultiplied by
      gn_w * rstd * WS, biases become host constants (vecs), and the
      fp8 projection inputs x8/y8 are cast on the host too.  No device
      stats, no device casts; the bf16 residual source x loads last.
    * all input DMAs issue from the SP HWDGE queue (a dma_start on the
      ACT/DVE queues blocks that sequencer ~1.5us each).
    * out-projection copy+bias+residual fused: either one DVE
      scalar_tensor_tensor (os = psum*OUT_SC + x_sb, bo pre-folded into
      x_sb by gpsimd) or an ACT copy plus a gpsimd residual add; output
      stores are bf16 and batched per q-block.
    * exp split ACT (accurate exp, fp8 out) / DVE (Schraudolph i8 bitcast)
      via a greedy balance with measured cost constants
      (ACT n*0.8333+185, DVE n*1.0417+125), reset at conveyor start.
      Softmax tails (reciprocal+mult; the BIR verifier rejects a fused
      tensor_tensor divide) stay on DVE and are counted in the balance.
  - big constant regions (q8 zero slot, vaug ones columns) are memset on
    Pool/DVE under the input DMA, split in needed-by order.
  - same conveyor skeleton as v2: 3-deep PSUM ring [128,2,512] for
    projections/scores/out, double-buffered [128,512] av accumulators,
    av lags 4 windows behind scores so the in-order PE never blocks the
    exp engines; the final q-block's out pieces run 512-wide on DVE with
    per-half stores to shorten the drain.
"""

import sys

sys.path.insert(0, "/opt/trn_rl_repo")

import numpy as np
import ml_dtypes

import concourse.bass as bass
import concourse.tile as tile
from concourse import bacc, mybir
from concourse.bass import ts
from concourse.bass_utils import run_bass_kernel_spmd

F32 = mybir.dt.float32
BF16 = mybir.dt.bfloat16
FP8 = mybir.dt.float8e4
I8 = mybir.dt.int8
AF = mybir.ActivationFunctionType
ALU = mybir.AluOpType
DR = mybir.MatmulPerfMode.DoubleRow
E4 = ml_dtypes.float8_e4m3
BF16_NP = ml_dtypes.bfloat16

N, C, L, H = 4, 512, 2048, 8
D, P = 64, 128
CO = C // P          # 4 channel chunks
LT = L // P          # 16 k-position tiles
QQ = 4               # 512-wide query chunks
QW = L // QQ
EPS = 1e-5
SCALE = float(D) ** -0.5

WS = 32.0            # host-side weight prescale (wq/wk/wv/wo)
QS = 32.0            # q fp8 scale (on top of SCALE)
KS = 4.0             # k fp8 scale
VS = 4.0             # v fp8 scale
AS = 64.0            # attn fp8 scale
ONEC = VS / AS       # ones-column value -> denominator lands pre-scaled
EXPS = 1.0 / (QS * KS)
K_SCH = 8.0 / np.log(2.0) * EXPS
B_SCH = 55.55        # calibrated against the real (round-to-nearest) path
OUT_SC = 1.0 / (WS * AS)


def _build_module():
    nc = bacc.Bacc("TRN2", target_bir_lowering=False, debug=False, num_devices=8)

    def din(name, shape, dt=F32):
        return nc.dram_tensor(name, list(shape), dt, kind="ExternalInput")

    x_d = din("x", (C, L), BF16)      # query-side input (residual only)
    x8_d = din("x8", (C, L), FP8)     # host-cast fp8 of x (projection input)
    y8_d = din("y8", (C, L), FP8)     # host-cast fp8 of y (k/v input)
    wq8_d = din("wq8", (C, C), FP8)   # ((w*gn_w*rstd).T * WS) fp8 [c_in,c_out]
    wk8_d = din("wk8", (C, C), FP8)
    wv8_d = din("wv8", (C, C), FP8)
    wo8_d = din("wo8", (C, C), FP8)
    # bq_eff*SCALE*QS, bk_eff*KS, bo_eff  (host-folded, see _core_inputs)
    vecs_d = din("vecs", (3 * C,))
    out_d = nc.dram_tensor("out", [C, L], BF16, kind="ExternalOutput")

    # build-time engine-load estimates (ns) for the greedy ACT/DVE split
    est = {"A": 0.0, "D": 0.0}

    def cost(eng, units):
        if eng == "A":
            return units * 0.8333 + 185.0
        return units * 1.0417 + 125.0

    def pick():
        return "A" if est["A"] <= est["D"] else "D"

    with tile.TileContext(nc) as tc:
        with (
            tc.tile_pool(name="persist", bufs=1) as pp,
            tc.tile_pool(name="small", bufs=1) as sp,
        ):
            x_sb = pp.tile([P, CO, L], BF16)     # 16K/part (residual source)
            x8 = pp.tile([P, CO, L], FP8)        # 8K  host-cast fp8 of x
            y8 = pp.tile([P, CO, L], FP8)        # 8K
            q8 = pp.tile([P, CO, 2, L], FP8)     # 16K (slot 1 = zeros)
            k8 = pp.tile([P, CO, L], FP8)        # 8K
            vaug = pp.tile([P, LT, H, P], FP8)   # 16K (cols 64:128 = ONEC)
            attn8 = pp.tile([P, CO, L], FP8)     # 8K
            wq8 = pp.tile([P, CO, C], FP8)       # 2K each
            wk8 = pp.tile([P, CO, C], FP8)
            wv8 = pp.tile([P, CO, C], FP8)
            wo8 = pp.tile([P, CO, C], FP8)

            vecs_pc = sp.tile([P, 3, CO], F32)
            bq_pc = vecs_pc[:, 0, :]
            bk_pc = vecs_pc[:, 1, :]
            bo_pc = vecs_pc[:, 2, :]

            # constant regions, under the input DMA (no data deps); DVE takes
            # only the late-needed vaug half (it must be free for k-copies by
            # ~6.5us), Pool interleaves the rest in needed-by order
            nc.vector.memset(vaug[:, LT // 2:LT, :, D:P], ONEC)
            nc.gpsimd.memset(vaug[:, 0:LT // 4, :, D:P], ONEC)
            nc.gpsimd.memset(q8[:, 0, 1, :], 0.0)
            nc.gpsimd.memset(q8[:, 1, 1, :], 0.0)
            nc.gpsimd.memset(vaug[:, LT // 4:LT // 2, :, D:P], ONEC)
            nc.gpsimd.memset(q8[:, 2, 1, :], 0.0)
            nc.gpsimd.memset(q8[:, 3, 1, :], 0.0)

            # ---- input DMAs, all on the SP HWDGE queue so the ACT/DVE
            # sequencers never block on DMA issue; fp8 projection inputs and
            # weights first (k-side before q-side), the bf16 residual source
            # last (first needed ~35us in)
            nc.sync.dma_start(
                vecs_pc[:], vecs_d[:].rearrange("(p t co) -> p t co", p=P, t=3))
            LH = L // 2
            y8_r = y8_d[:].rearrange("(co p) l -> p co l", p=P)
            x8_r = x8_d[:].rearrange("(co p) l -> p co l", p=P)
            # column-halved input loads: each projection contracts all
            # channels but only for its own columns, so the first half of
            # k/q-proj can start as soon as its half-tensor lands
            nc.sync.dma_start(wk8[:], wk8_d[:].rearrange("(ko p) o -> p ko o", p=P))
            nc.sync.dma_start(y8[:, :, 0:LH], y8_r[:, :, 0:LH])
            nc.sync.dma_start(wv8[:], wv8_d[:].rearrange("(ko p) o -> p ko o", p=P))
            nc.sync.dma_start(y8[:, :, LH:L], y8_r[:, :, LH:L])
            nc.sync.dma_start(x8[:, :, 0:LH], x8_r[:, :, 0:LH])
            nc.sync.dma_start(wq8[:], wq8_d[:].rearrange("(ko p) o -> p ko o", p=P))
            nc.sync.dma_start(x8[:, :, LH:L], x8_r[:, :, LH:L])
            nc.sync.dma_start(wo8[:], wo8_d[:].rearrange("(ko p) o -> p ko o", p=P))
            nc.sync.dma_start(x_sb[:], x_d[:].rearrange("(co p) l -> p co l", p=P))

            # fold output bias into the residual source (gpsimd, off stream)
            for mo in range(CO):
                nc.gpsimd.tensor_scalar(x_sb[:, mo, :], x_sb[:, mo, :],
                                        bo_pc[:, mo:mo + 1], 0.0,
                                        op0=ALU.add, op1=ALU.add)

            # ================= conveyor: proj -> attention -> out-proj ===
            # DVE's vaug memset is only partly hidden by its idle lead-in
            est["A"] = 0.0
            est["D"] = 2000.0
            with (
                tc.tile_pool(name="ring", bufs=3, space="PSUM") as rsp,
                tc.tile_pool(name="oh", bufs=2, space="PSUM") as ohp,
                tc.tile_pool(name="ptp", bufs=10) as ptp,
                tc.tile_pool(name="rpool", bufs=3) as rp,
                tc.tile_pool(name="opool", bufs=3) as op_,
                tc.tile_pool(name="ospool", bufs=3) as osp,
            ):
                def take2():
                    rt = rsp.tile([P, 2, QW], F32, tag="ring")
                    return rt

                def psum_copy_scale_bias(dst, src, scale_imm, bias_ap, units):
                    """dst = src*scale + bias via ACT or DVE (greedy)."""
                    eng = pick()
                    est[eng] += cost(eng, units)
                    if eng == "A":
                        nc.scalar.activation(dst, src, AF.Identity,
                                             bias=bias_ap, scale=scale_imm)
                    else:
                        nc.vector.tensor_scalar(dst, src, scale_imm, bias_ap,
                                                op0=ALU.mult, op1=ALU.add)

                def psum_copy_scale(dst, src, scale_imm, units):
                    eng = pick()
                    est[eng] += cost(eng, units)
                    if eng == "A":
                        nc.scalar.mul(dst, src, scale_imm)
                    else:
                        nc.vector.tensor_scalar(dst, src, scale_imm, 0.0,
                                                op0=ALU.mult, op1=ALU.add)

                def emit_kq(side, p, lc2):
                    rt = take2()
                    w8 = wk8 if side == "k" else wq8
                    src = y8 if side == "k" else x8
                    for j in range(2):
                        lc = 2 * lc2 + j
                        for m in range(2):
                            nc.tensor.matmul(
                                rt[:, j, :],
                                w8[:, 2 * m:2 * m + 2, ts(p, P)],
                                src[:, 2 * m:2 * m + 2, ts(lc, QW)],
                                start=(m == 0), stop=(m == 1), perf_mode=DR)
                    if side == "k":
                        dst = k8[:, p, 2 * lc2 * QW:(2 * lc2 + 2) * QW]
                        dst = dst.rearrange("p (a b) -> p a b", a=2)
                        psum_copy_scale_bias(dst, rt[:], KS / WS,
                                             bk_pc[:, p:p + 1], 1024)
                    else:
                        dst = q8[:, p, 0, 2 * lc2 * QW:(2 * lc2 + 2) * QW]
                        dst = dst.rearrange("p (a b) -> p a b", a=2)
                        psum_copy_scale_bias(dst, rt[:], SCALE * QS / WS,
                                             bq_pc[:, p:p + 1], 1024)

                def emit_vp(lt2):
                    rt = take2()
                    for i in range(2):
                        lt = 2 * lt2 + i
                        for m in range(2):
                            nc.tensor.matmul(
                                rt[:, i, :],
                                y8[:, 2 * m:2 * m + 2, ts(lt, P)],
                                wv8[:, 2 * m:2 * m + 2, :],
                                start=(m == 0), stop=(m == 1), perf_mode=DR)
                    dst = vaug[:, 2 * lt2:2 * lt2 + 2, :, 0:D]
                    src = rt[:].rearrange("p a (h d) -> p a h d", d=D)
                    psum_copy_scale(dst, src, VS / WS, 1024)

                oh_cur = {}

                def emit_attn_scores(qq, p, h, kt2):
                    rt = take2()
                    lo = D * h
                    qs = qq * QW
                    for j in range(2):
                        kt = 2 * kt2 + j
                        lhsT = (k8[lo:lo + D, p, ts(kt, P)]
                                .unsqueeze(1).broadcast_to([D, 2, P]))
                        nc.tensor.matmul(rt[:, j, :], lhsT,
                                         q8[lo:lo + D, p, :, qs:qs + QW],
                                         start=True, stop=True, perf_mode=DR)
                    return rt

                def emit_exp(rt):
                    pt_t = ptp.tile([P, 2, QW], FP8, tag="pt")
                    eng = pick()
                    est[eng] += cost(eng, 2 * QW)
                    if eng == "A":
                        nc.scalar.activation(pt_t[:], rt[:],
                                             AF.Exp, bias=0.0, scale=EXPS)
                    else:
                        nc.vector.tensor_scalar(
                            pt_t[:].bitcast(I8), rt[:], K_SCH, B_SCH,
                            op0=ALU.mult, op1=ALU.add)
                    return pt_t

                pend_tails = []

                def emit_attn_av(qq, p, h, kt2, pt_t):
                    if kt2 == 0:
                        oh_t = ohp.tile([P, QW], F32, tag="oh")
                        oh_cur[h] = oh_t
                    oh = oh_cur[h]
                    nc.tensor.matmul(oh[:], vaug[:, 2 * kt2:2 * kt2 + 2, h, :],
                                     pt_t[:],
                                     start=(kt2 == 0), stop=(kt2 == 7),
                                     perf_mode=DR)
                    if kt2 == 7:
                        # defer the tail: emitting it now would park a
                        # PE-gated op at the head of DVE's in-order queue,
                        # stalling ready exp work queued behind it
                        pend_tails.append((qq, p, h, oh))

                def emit_tail(qq, p, h, oh):
                    # tail: r = 1/den ; attn8 = num * r  (DVE only; the
                    # BIR verifier rejects tensor_tensor divide)
                    qs = qq * QW
                    lo = D * h
                    r = rp.tile([D, QW], F32, tag="r")
                    nc.vector.reciprocal(r[:], oh[D:P, :])
                    nc.vector.tensor_tensor(attn8[lo:lo + D, p, qs:qs + QW],
                                            oh[0:D, :], r[:], ALU.mult)
                    est["D"] += 2 * cost("D", QW)

                def flush_tails(n=0):
                    while len(pend_tails) > n:
                        emit_tail(*pend_tails.pop(0))

                os_cur = {}

                def emit_out(qq, mo2):
                    rt = take2()
                    qs = qq * QW
                    for i in range(2):
                        mo = 2 * mo2 + i
                        for m in range(2):
                            nc.tensor.matmul(
                                rt[:, i, :],
                                wo8[:, 2 * m:2 * m + 2, ts(mo, P)],
                                attn8[:, 2 * m:2 * m + 2, qs:qs + QW],
                                start=(m == 0), stop=(m == 1), perf_mode=DR)
                    if mo2 == 0:
                        os_t = osp.tile([P, CO, QW], BF16, tag="os")
                        os_cur[qq] = os_t
                    os_t = os_cur[qq]

                    def one_copy(sl, units, force=None):
                        dst = os_t[:, 2 * mo2:2 * mo2 + 2, :][:, sl, :]
                        src = rt[:, sl, :]
                        xsl = x_sb[:, 2 * mo2:2 * mo2 + 2, qs:qs + QW][:, sl, :]
                        eng = force or pick()
                        est[eng] += cost(eng, units)
                        if eng == "D":
                            # fused copy+residual: os = psum*OUT_SC + x_sb
                            nc.vector.scalar_tensor_tensor(
                                dst, src, OUT_SC, xsl,
                                op0=ALU.mult, op1=ALU.add)
                        else:
                            # ACT copy + residual add on the idle Pool engine
                            ot = op_.tile([P, 2, QW], F32, tag="ot")
                            nc.scalar.mul(ot[:, sl, :], src, OUT_SC)
                            nc.gpsimd.tensor_tensor(dst, ot[:, sl, :], xsl,
                                                    ALU.add)

                    if qq == QQ - 1:
                        # drain: 512-col pieces; slow ACT+Pool path first,
                        # fused DVE path last; store each half as it's done
                        one_copy(slice(0, 1), 512, "D")
                        one_copy(slice(1, 2), 512, "D")
                        nc.sync.dma_start(
                            out_d[:].rearrange("(mo p) l -> p mo l", p=P)
                            [:, 2 * mo2:2 * mo2 + 2, qs:qs + QW],
                            os_t[:, 2 * mo2:2 * mo2 + 2, :])
                    else:
                        one_copy(slice(0, 2), 1024)
                        if mo2 == 1:
                            nc.sync.dma_start(
                                out_d[:].rearrange("(mo p) l -> p mo l", p=P)
                                [:, :, qs:qs + QW], os_t[:])

                def emit_out_final(qq):
                    # final q-block: run every start-pass (channels 0:256,
                    # whose tails finish early) before any stop-pass, so the
                    # in-order PE does the m0 work while the last tails run
                    qs = qq * QW
                    rts = [take2(), take2()]
                    for m in range(2):
                        for mo2 in range(2):
                            for i in range(2):
                                mo = 2 * mo2 + i
                                nc.tensor.matmul(
                                    rts[mo2][:, i, :],
                                    wo8[:, 2 * m:2 * m + 2, ts(mo, P)],
                                    attn8[:, 2 * m:2 * m + 2, qs:qs + QW],
                                    start=(m == 0), stop=(m == 1),
                                    perf_mode=DR)
                    os_t = osp.tile([P, CO, QW], BF16, tag="os")
                    for mo2 in range(2):
                        for i in range(2):
                            dst = os_t[:, 2 * mo2 + i, :].unsqueeze(1)
                            src = rts[mo2][:, i, :].unsqueeze(1)
                            xsl = x_sb[:, 2 * mo2 + i, qs:qs + QW].unsqueeze(1)
                            if mo2 == 0 and i == 0:
                                # one piece via idle ACT+Pool so only three
                                # copies serialize on DVE after the last tail
                                ot = op_.tile([P, 1, QW], F32, tag="otf")
                                nc.scalar.mul(ot[:], src, OUT_SC)
                                nc.gpsimd.tensor_tensor(dst, ot[:], xsl,
                                                        ALU.add)
                            else:
                                est["D"] += cost("D", 512)
                                nc.vector.scalar_tensor_tensor(
                                    dst, src, OUT_SC, xsl,
                                    op0=ALU.mult, op1=ALU.add)
                        nc.sync.dma_start(
                            out_d[:].rearrange("(mo p) l -> p mo l", p=P)
                            [:, 2 * mo2:2 * mo2 + 2, qs:qs + QW],
                            os_t[:, 2 * mo2:2 * mo2 + 2, :])

                # ---- window stream construction (k first: wk lands first) --
                stream = []
                for lc2 in range(2):
                    stream.append(("kq", "k", 0, lc2))
                stream.append(("vp", 0))
                stream.append(("vp", 1))
                for lc2 in range(2):
                    stream.append(("kq", "q", 0, lc2))
                for qq in range(QQ):
                    for p in range(CO):
                        inter = []
                        if qq == 0 and p < 3:
                            inter = [("kq", side, p + 1, l)
                                     for side in ("k", "q") for l in range(2)]
                        if qq >= 1 and p == 0:
                            inter = [("out", qq - 1, m) for m in range(2)]
                        atw = []
                        for h in range(2):
                            for kt2 in range(8):
                                if qq == 0 and p == 0 and h == 0 and kt2 >= 2:
                                    atw.append(("vp", kt2))
                                atw.append(("attn", qq, p, h, kt2))
                        # spread `inter` evenly through the attention run
                        out2 = []
                        k = 0
                        step = max(2, (len(atw) - 1) // max(1, len(inter)))
                        for i, w in enumerate(atw):
                            out2.append(w)
                            if inter and k < len(inter) and i % step == step - 1:
                                out2.append(inter[k])
                                k += 1
                        out2.extend(inter[k:])
                        stream.extend(out2)
                stream.append(("outfinal", QQ - 1))

                # ---- emission, software-pipelined (av lags scores) ----
                pend = []

                def flush(n=0):
                    while len(pend) > n:
                        emit_attn_av(*pend.pop(0))

                for w in stream:
                    if w[0] == "kq":
                        emit_kq(w[1], w[2], w[3])
                    elif w[0] == "vp":
                        emit_vp(w[1])
                    elif w[0] == "out":
                        # out-proj reads attn8 written by pending tails
                        flush()
                        flush_tails()
                        emit_out(w[1], w[2])
                    elif w[0] == "outfinal":
                        flush()
                        flush_tails()
                        emit_out_final(w[1])
                    else:
                        rt = emit_attn_scores(*w[1:])
                        pt_t = emit_exp(rt)
                        flush(4)
                        flush_tails(0)
                        pend.append((*w[1:], pt_t))
                flush()
                flush_tails()

    nc.compile()
    return nc


_NC_CACHE = None


def _get_module():
    global _NC_CACHE
    if _NC_CACHE is None:
        _NC_CACHE = _build_module()
    return _NC_CACHE


def _core_inputs(x, y, gnx_w, gnx_b, gny_w, gny_b, qw_q, qb_q, qw_kv, qb_kv,
                 ow, ob):
    f32 = lambda a: np.asarray(a, np.float32)
    x, y = f32(x), f32(y)
    wq, bq = f32(qw_q[0:C]), f32(qb_q[0:C])
    wk, bk = f32(qw_kv[C:2 * C]), f32(qb_kv[C:2 * C])
    wv, bv = f32(qw_kv[2 * C:3 * C]), f32(qb_kv[2 * C:3 * C])
    ow, ob = f32(ow), f32(ob)
    gnx_w, gnx_b = f32(gnx_w), f32(gnx_b)
    gny_w, gny_b = f32(gny_w), f32(gny_b)
    # exact GroupNorm(1,C) stats on the host, folded into weights + biases
    mu_x, rstd_x = x.mean(), 1.0 / np.sqrt(x.var() + EPS)
    mu_y, rstd_y = y.mean(), 1.0 / np.sqrt(y.var() + EPS)
    f8 = lambda w, g, rstd: np.ascontiguousarray(
        (w * (g * rstd)[None, :]).T * WS).astype(E4)
    bq_eff = bq + wq @ (gnx_b - mu_x * rstd_x * gnx_w)
    bk_eff = bk + wk @ (gny_b - mu_y * rstd_y * gny_w)
    bv_eff = bv + wv @ (gny_b - mu_y * rstd_y * gny_w)
    bo_eff = ob + ow @ bv_eff
    # partition-major layout: one contiguous 48B run per partition keeps
    # the vecs DMA at 128 descriptors instead of 1536 7ns-minimum ones
    vecs = np.ascontiguousarray(
        np.stack([bq_eff * SCALE * QS, bk_eff * KS, bo_eff])
        .reshape(3, CO, P).transpose(2, 0, 1)).ravel()
    return {
        "x": np.ascontiguousarray(x).astype(BF16_NP),
        "x8": np.ascontiguousarray(x).astype(E4),
        "y8": np.ascontiguousarray(y).astype(E4),
        "wq8": f8(wq, gnx_w, rstd_x), "wk8": f8(wk, gny_w, rstd_y),
        "wv8": f8(wv, gny_w, rstd_y),
        "wo8": np.ascontiguousarray(ow.T * WS).astype(E4),
        "vecs": vecs,
    }


def kernel(a, b, gn_a_w, gn_a_b, gn_b_w, gn_b_b,
           qkv_a_w, qkv_a_b, qkv_b_w, qkv_b_b,
           out_a_w, out_a_b, out_b_w, out_b_b):
    a = np.asarray(a); b = np.asarray(b)
    nc = _get_module()
    in_maps = []
    for s in range(N):
        # direction a->b : q from a, k/v from b, output -> out_a[s]
        in_maps.append(_core_inputs(a[s], b[s], gn_a_w, gn_a_b, gn_b_w, gn_b_b,
                                    qkv_a_w, qkv_a_b, qkv_b_w, qkv_b_b,
                                    out_a_w, out_a_b))
        # direction b->a : q from b, k/v from a, output -> out_b[s]
        in_maps.append(_core_inputs(b[s], a[s], gn_b_w, gn_b_b, gn_a_w, gn_a_b,
                                    qkv_b_w, qkv_b_b, qkv_a_w, qkv_a_b,
                                    out_b_w, out_b_b))
    res = run_bass_kernel_spmd(nc, in_maps, core_ids=list(range(2 * N)))
    out_a = np.stack([np.asarray(res.results[2 * s]["out"]).view(BF16_NP)
                      if res.results[2 * s]["out"].dtype != BF16_NP
                      else res.results[2 * s]["out"] for s in range(N)])
    out_b = np.stack([np.asarray(res.results[2 * s + 1]["out"]).view(BF16_NP)
                      if res.results[2 * s + 1]["out"].dtype != BF16_NP
                      else res.results[2 * s + 1]["out"] for s in range(N)])
    return out_a.astype(np.float32), out_b.astype(np.float32)


# revision 67
# speedup vs baseline: 1.0024x; 1.0016x over previous
"""Cross-attention 1d kernel for Trainium2 (Bass/Tile), SPMD over 8 NeuronCores.

Problem (hardcoded shapes): N=4, C=512, L=2048, H=8, D=64.
  out_a = out_a_w @ attn(a_norm -> b_norm) + out_a_b + a
  out_b = out_b_w @ attn(b_norm -> a_norm) + out_b_b + b

Sharding: 8 cores = 4 samples x 2 directions (a->b, b->a). Each core computes
one full [512, 2048] output tensor. No cross-core communication.

v3 design notes (vs v2):
  - ACT+DVE are the bottleneck: every PSUM->SBUF crossing must use them
    (gpsimd is BIR-forbidden from PSUM), and the softmax exp alone is 262144
    128-row columns.  All changes cut their load or idle time:
    * GroupNorm computed on the HOST (exact mean/var of the full input) and
      folded into the projections: weights are premultiplied by
      gn_w * rstd * WS, biases become host constants (vecs), and the
      fp8 projection inputs x8/y8 are cast on the host too.  No device
      stats, no device casts; the bf16 residual source x loads last.
    * all input DMAs issue from the SP HWDGE queue (a dma_start on the
      ACT/DVE queues blocks that sequencer ~1.5us each).
    * out-projection copy+bias+residual fused: either one DVE
      scalar_tensor_tensor (os = psum*OUT_SC + x_sb, bo pre-folded into
      x_sb by gpsimd) or an ACT copy plus a gpsimd residual add; output
      stores are bf16 and batched per q-block.
    * exp split ACT (accurate exp, fp8 out) / DVE (Schraudolph i8 bitcast)
      via a greedy balance with measured cost constants
      (ACT n*0.8333+185, DVE n*1.0417+125), reset at conveyor start.
      Softmax tails (reciprocal+mult; the BIR verifier rejects a fused
      tensor_tensor divide) stay on DVE and are counted in the balance.
  - big constant regions (q8 zero slot, vaug ones columns) are memset on
    Pool/DVE under the input DMA, split in needed-by order.
  - same conveyor skeleton as v2: 3-deep PSUM ring [128,2,512] for
    projections/scores/out, double-buffered [128,512] av accumulators,
    av lags 4 windows behind scores so the in-order PE never blocks the
    exp engines; the final q-block's out pieces run 512-wide on DVE with
    per-half stores to shorten the drain.
"""

import sys

sys.path.insert(0, "/opt/trn_rl_repo")

import numpy as np
import ml_dtypes

import concourse.bass as bass
import concourse.tile as tile
from concourse import bacc, mybir
from concourse.bass import ts
from concourse.bass_utils import run_bass_kernel_spmd

F32 = mybir.dt.float32
BF16 = mybir.dt.bfloat16
FP8 = mybir.dt.float8e4
I8 = mybir.dt.int8
AF = mybir.ActivationFunctionType
ALU = mybir.AluOpType
DR = mybir.MatmulPerfMode.DoubleRow
E4 = ml_dtypes.float8_e4m3
BF16_NP = ml_dtypes.bfloat16

N, C, L, H = 4, 512, 2048, 8
D, P = 64, 128
CO = C // P          # 4 channel chunks
LT = L // P          # 16 k-position tiles
QQ = 4               # 512-wide query chunks
QW = L // QQ
EPS = 1e-5
SCALE = float(D) ** -0.5

WS = 32.0            # host-side weight prescale (wq/wk/wv/wo)
QS = 32.0            # q fp8 scale (on top of SCALE)
KS = 4.0             # k fp8 scale
VS = 4.0             # v fp8 scale
AS = 64.0            # attn fp8 scale
ONEC = VS / AS       # ones-column value -> denominator lands pre-scaled
EXPS = 1.0 / (QS * KS)
K_SCH = 8.0 / np.log(2.0) * EXPS
B_SCH = 55.55        # calibrated against the real (round-to-nearest) path
OUT_SC = 1.0 / (WS * AS)


def _build_module():
    nc = bacc.Bacc("TRN2", target_bir_lowering=False, debug=False, num_devices=8)

    def din(name, shape, dt=F32):
        return nc.dram_tensor(name, list(shape), dt, kind="ExternalInput")

    x_d = din("x", (C, L), BF16)      # query-side input (residual only)
    x8_d = din("x8", (C, L), FP8)     # host-cast fp8 of x (projection input)
    y8_d = din("y8", (C, L), FP8)     # host-cast fp8 of y (k/v input)
    wq8_d = din("wq8", (C, C), FP8)   # ((w*gn_w*rstd).T * WS) fp8 [c_in,c_out]
    wk8_d = din("wk8", (C, C), FP8)
    wv8_d = din("wv8", (C, C), FP8)
    wo8_d = din("wo8", (C, C), FP8)
    # bq_eff*SCALE*QS, bk_eff*KS, bo_eff  (host-folded, see _core_inputs)
    vecs_d = din("vecs", (3 * C,))
    out_d = nc.dram_tensor("out", [C, L], BF16, kind="ExternalOutput")

    # build-time engine-load estimates (ns) for the greedy ACT/DVE split
    est = {"A": 0.0, "D": 0.0}

    def cost(eng, units):
        if eng == "A":
            return units * 0.8333 + 185.0
        return units * 1.0417 + 125.0

    def pick():
        return "A" if est["A"] <= est["D"] else "D"

    with tile.TileContext(nc) as tc:
        with (
            tc.tile_pool(name="persist", bufs=1) as pp,
            tc.tile_pool(name="small", bufs=1) as sp,
        ):
            x_sb = pp.tile([P, CO, L], BF16)     # 16K/part (residual source)
            x8 = pp.tile([P, CO, L], FP8)        # 8K  host-cast fp8 of x
            y8 = pp.tile([P, CO, L], FP8)        # 8K
            q8 = pp.tile([P, CO, 2, L], FP8)     # 16K (slot 1 = zeros)
            k8 = pp.tile([P, CO, L], FP8)        # 8K
            vaug = pp.tile([P, LT, H, P], FP8)   # 16K (cols 64:128 = ONEC)
            attn8 = pp.tile([P, CO, L], FP8)     # 8K
            wq8 = pp.tile([P, CO, C], FP8)       # 2K each
            wk8 = pp.tile([P, CO, C], FP8)
            wv8 = pp.tile([P, CO, C], FP8)
            wo8 = pp.tile([P, CO, C], FP8)

            vecs_pc = sp.tile([P, 3, CO], F32)
            bq_pc = vecs_pc[:, 0, :]
            bk_pc = vecs_pc[:, 1, :]
            bo_pc = vecs_pc[:, 2, :]

            # constant regions, under the input DMA (no data deps); DVE takes
            # only the late-needed vaug half (it must be free for k-copies by
            # ~6.5us), Pool interleaves the rest in needed-by order
            nc.vector.memset(vaug[:, LT // 2:LT, :, D:P], ONEC)
            nc.gpsimd.memset(vaug[:, 0:LT // 4, :, D:P], ONEC)
            nc.gpsimd.memset(q8[:, 0, 1, :], 0.0)
            nc.gpsimd.memset(q8[:, 1, 1, :], 0.0)
            nc.gpsimd.memset(vaug[:, LT // 4:LT // 2, :, D:P], ONEC)
            nc.gpsimd.memset(q8[:, 2, 1, :], 0.0)
            nc.gpsimd.memset(q8[:, 3, 1, :], 0.0)

            # ---- input DMAs, all on the SP HWDGE queue so the ACT/DVE
            # sequencers never block on DMA issue; fp8 projection inputs and
            # weights first (k-side before q-side), the bf16 residual source
            # last (first needed ~35us in)
            nc.sync.dma_start(
                vecs_pc[:], vecs_d[:].rearrange("(p t co) -> p t co", p=P, t=3))
            LH = L // 2
            y8_r = y8_d[:].rearrange("(co p) l -> p co l", p=P)
            x8_r = x8_d[:].rearrange("(co p) l -> p co l", p=P)
            # column-halved input loads: each projection contracts all
            # channels but only for its own columns, so the first half of
            # k/q-proj can start as soon as its half-tensor lands
            nc.sync.dma_start(wk8[:], wk8_d[:].rearrange("(ko p) o -> p ko o", p=P))
            nc.sync.dma_start(y8[:, :, 0:LH], y8_r[:, :, 0:LH])
            nc.sync.dma_start(wv8[:], wv8_d[:].rearrange("(ko p) o -> p ko o", p=P))
            nc.sync.dma_start(y8[:, :, LH:L], y8_r[:, :, LH:L])
            nc.sync.dma_start(x8[:, :, 0:LH], x8_r[:, :, 0:LH])
            nc.sync.dma_start(wq8[:], wq8_d[:].rearrange("(ko p) o -> p ko o", p=P))
            nc.sync.dma_start(x8[:, :, LH:L], x8_r[:, :, LH:L])
            nc.sync.dma_start(wo8[:], wo8_d[:].rearrange("(ko p) o -> p ko o", p=P))
            nc.sync.dma_start(x_sb[:], x_d[:].rearrange("(co p) l -> p co l", p=P))

            # fold output bias into the residual source (gpsimd, off stream)
            for mo in range(CO):
                nc.gpsimd.tensor_scalar(x_sb[:, mo, :], x_sb[:, mo, :],
                                        bo_pc[:, mo:mo + 1], 0.0,
                                        op0=ALU.add, op1=ALU.add)

            # ================= conveyor: proj -> attention -> out-proj ===
            # DVE's vaug memset is only partly hidden by its idle lead-in
            est["A"] = 0.0
            est["D"] = 2000.0
            with (
                tc.tile_pool(name="ring", bufs=3, space="PSUM") as rsp,
                tc.tile_pool(name="oh", bufs=2, space="PSUM") as ohp,
                tc.tile_pool(name="ptp", bufs=10) as ptp,
                tc.tile_pool(name="rpool", bufs=3) as rp,
                tc.tile_pool(name="opool", bufs=3) as op_,
                tc.tile_pool(name="ospool", bufs=3) as osp,
            ):
                def take2():
                    rt = rsp.tile([P, 2, QW], F32, tag="ring")
                    return rt

                def psum_copy_scale_bias(dst, src, scale_imm, bias_ap, units):
                    """dst = src*scale + bias via ACT or DVE (greedy)."""
                    eng = pick()
                    est[eng] += cost(eng, units)
                    if eng == "A":
                        nc.scalar.activation(dst, src, AF.Identity,
                                             bias=bias_ap, scale=scale_imm)
                    else:
                        nc.vector.tensor_scalar(dst, src, scale_imm, bias_ap,
                                                op0=ALU.mult, op1=ALU.add)

                def psum_copy_scale(dst, src, scale_imm, units):
                    eng = pick()
                    est[eng] += cost(eng, units)
                    if eng == "A":
                        nc.scalar.mul(dst, src, scale_imm)
                    else:
                        nc.vector.tensor_scalar(dst, src, scale_imm, 0.0,
                                                op0=ALU.mult, op1=ALU.add)

                def emit_kq(side, p, lc2):
                    rt = take2()
                    w8 = wk8 if side == "k" else wq8
                    src = y8 if side == "k" else x8
                    for j in range(2):
                        lc = 2 * lc2 + j
                        for m in range(2):
                            nc.tensor.matmul(
                                rt[:, j, :],
                                w8[:, 2 * m:2 * m + 2, ts(p, P)],
                                src[:, 2 * m:2 * m + 2, ts(lc, QW)],
                                start=(m == 0), stop=(m == 1), perf_mode=DR)
                    if side == "k":
                        dst = k8[:, p, 2 * lc2 * QW:(2 * lc2 + 2) * QW]
                        dst = dst.rearrange("p (a b) -> p a b", a=2)
                        psum_copy_scale_bias(dst, rt[:], KS / WS,
                                             bk_pc[:, p:p + 1], 1024)
                    else:
                        dst = q8[:, p, 0, 2 * lc2 * QW:(2 * lc2 + 2) * QW]
                        dst = dst.rearrange("p (a b) -> p a b", a=2)
                        psum_copy_scale_bias(dst, rt[:], SCALE * QS / WS,
                                             bq_pc[:, p:p + 1], 1024)

                def emit_vp(lt2):
                    rt = take2()
                    for i in range(2):
                        lt = 2 * lt2 + i
                        for m in range(2):
                            nc.tensor.matmul(
                                rt[:, i, :],
                                y8[:, 2 * m:2 * m + 2, ts(lt, P)],
                                wv8[:, 2 * m:2 * m + 2, :],
                                start=(m == 0), stop=(m == 1), perf_mode=DR)
                    dst = vaug[:, 2 * lt2:2 * lt2 + 2, :, 0:D]
                    src = rt[:].rearrange("p a (h d) -> p a h d", d=D)
                    psum_copy_scale(dst, src, VS / WS, 1024)

                oh_cur = {}

                def emit_attn_scores(qq, p, h, kt2):
                    rt = take2()
                    lo = D * h
                    qs = qq * QW
                    for j in range(2):
                        kt = 2 * kt2 + j
                        lhsT = (k8[lo:lo + D, p, ts(kt, P)]
                                .unsqueeze(1).broadcast_to([D, 2, P]))
                        nc.tensor.matmul(rt[:, j, :], lhsT,
                                         q8[lo:lo + D, p, :, qs:qs + QW],
                                         start=True, stop=True, perf_mode=DR)
                    return rt

                def emit_exp(rt):
                    pt_t = ptp.tile([P, 2, QW], FP8, tag="pt")
                    eng = pick()
                    est[eng] += cost(eng, 2 * QW)
                    if eng == "A":
                        nc.scalar.activation(pt_t[:], rt[:],
                                             AF.Exp, bias=0.0, scale=EXPS)
                    else:
                        nc.vector.tensor_scalar(
                            pt_t[:].bitcast(I8), rt[:], K_SCH, B_SCH,
                            op0=ALU.mult, op1=ALU.add)
                    return pt_t

                pend_tails = []

                def emit_attn_av(qq, p, h, kt2, pt_t):
                    if kt2 == 0:
                        oh_t = ohp.tile([P, QW], F32, tag="oh")
                        oh_cur[h] = oh_t
                    oh = oh_cur[h]
                    nc.tensor.matmul(oh[:], vaug[:, 2 * kt2:2 * kt2 + 2, h, :],
                                     pt_t[:],
                                     start=(kt2 == 0), stop=(kt2 == 7),
                                     perf_mode=DR)
                    if kt2 == 7:
                        # defer the tail: emitting it now would park a
                        # PE-gated op at the head of DVE's in-order queue,
                        # stalling ready exp work queued behind it
                        pend_tails.append((qq, p, h, oh))

                def emit_tail(qq, p, h, oh):
                    # tail: r = 1/den ; attn8 = num * r  (DVE only; the
                    # BIR verifier rejects tensor_tensor divide)
                    qs = qq * QW
                    lo = D * h
                    r = rp.tile([D, QW], F32, tag="r")
                    nc.vector.reciprocal(r[:], oh[D:P, :])
                    nc.vector.tensor_tensor(attn8[lo:lo + D, p, qs:qs + QW],
                                            oh[0:D, :], r[:], ALU.mult)
                    est["D"] += 2 * cost("D", QW)

                def flush_tails(n=0):
                    while len(pend_tails) > n:
                        emit_tail(*pend_tails.pop(0))

                os_cur = {}

                def emit_out(qq, mo2):
                    rt = take2()
                    qs = qq * QW
                    for i in range(2):
                        mo = 2 * mo2 + i
                        for m in range(2):
                            nc.tensor.matmul(
                                rt[:, i, :],
                                wo8[:, 2 * m:2 * m + 2, ts(mo, P)],
                                attn8[:, 2 * m:2 * m + 2, qs:qs + QW],
                                start=(m == 0), stop=(m == 1), perf_mode=DR)
                    if mo2 == 0:
                        os_t = osp.tile([P, CO, QW], BF16, tag="os")
                        os_cur[qq] = os_t
                    os_t = os_cur[qq]

                    def one_copy(sl, units, force=None):
                        dst = os_t[:, 2 * mo2:2 * mo2 + 2, :][:, sl, :]
                        src = rt[:, sl, :]
                        xsl = x_sb[:, 2 * mo2:2 * mo2 + 2, qs:qs + QW][:, sl, :]
                        eng = force or pick()
                        est[eng] += cost(eng, units)
                        if eng == "D":
                            # fused copy+residual: os = psum*OUT_SC + x_sb
                            nc.vector.scalar_tensor_tensor(
                                dst, src, OUT_SC, xsl,
                                op0=ALU.mult, op1=ALU.add)
                        else:
                            # ACT copy + residual add on the idle Pool engine
                            ot = op_.tile([P, 2, QW], F32, tag="ot")
                            nc.scalar.mul(ot[:, sl, :], src, OUT_SC)
                            nc.gpsimd.tensor_tensor(dst, ot[:, sl, :], xsl,
                                                    ALU.add)

                    if qq == QQ - 1:
                        # drain: 512-col pieces; slow ACT+Pool path first,
                        # fused DVE path last; store each half as it's done
                        one_copy(slice(0, 1), 512, "D")
                        one_copy(slice(1, 2), 512, "D")
                        nc.sync.dma_start(
                            out_d[:].rearrange("(mo p) l -> p mo l", p=P)
                            [:, 2 * mo2:2 * mo2 + 2, qs:qs + QW],
                            os_t[:, 2 * mo2:2 * mo2 + 2, :])
                    else:
                        one_copy(slice(0, 2), 1024)
                        if mo2 == 1:
                            nc.sync.dma_start(
                                out_d[:].rearrange("(mo p) l -> p mo l", p=P)
                                [:, :, qs:qs + QW], os_t[:])

                def emit_out_final(qq):
                    # final q-block: run every start-pass (channels 0:256,
                    # whose tails finish early) before any stop-pass, so the
                    # in-order PE does the m0 work while the last tails run
                    qs = qq * QW
                    rts = [take2(), take2()]
                    for m in range(2):
                        for mo2 in range(2):
                            for i in range(2):
                                mo = 2 * mo2 + i
                                nc.tensor.matmul(
                                    rts[mo2][:, i, :],
                                    wo8[:, 2 * m:2 * m + 2, ts(mo, P)],
                                    attn8[:, 2 * m:2 * m + 2, qs:qs + QW],
                                    start=(m == 0), stop=(m == 1),
                                    perf_mode=DR)
                    os_t = osp.tile([P, CO, QW], BF16, tag="os")
                    for mo2 in range(2):
                        for i in range(2):
                            dst = os_t[:, 2 * mo2 + i, :].unsqueeze(1)
                            src = rts[mo2][:, i, :].unsqueeze(1)
                            xsl = x_sb[:, 2 * mo2 + i, qs:qs + QW].unsqueeze(1)
                            if mo2 == 1 and i == 0:
                                # one piece via idle ACT+Pool so only three
                                # copies serialize on DVE after the last tail
                                ot = op_.tile([P, 1, QW], F32, tag="otf")
                                nc.scalar.mul(ot[:], src, OUT_SC)
                                nc.gpsimd.tensor_tensor(dst, ot[:], xsl,
                                                        ALU.add)
                            else:
                                est["D"] += cost("D", 512)
                                nc.vector.scalar_tensor_tensor(
                                    dst, src, OUT_SC, xsl,
                                    op0=ALU.mult, op1=ALU.add)
                        nc.sync.dma_start(
                            out_d[:].rearrange("(mo p) l -> p mo l", p=P)
                            [:, 2 * mo2:2 * mo2 + 2, qs:qs + QW],
                            os_t[:, 2 * mo2:2 * mo2 + 2, :])

                # ---- window stream construction (k first: wk lands first) --
                stream = []
                for lc2 in range(2):
                    stream.append(("kq", "k", 0, lc2))
                stream.append(("vp", 0))
                stream.append(("vp", 1))
                for lc2 in range(2):
                    stream.append(("kq", "q", 0, lc2))
                for qq in range(QQ):
                    for p in range(CO):
                        inter = []
                        if qq == 0 and p < 3:
                            inter = [("kq", side, p + 1, l)
                                     for side in ("k", "q") for l in range(2)]
                        if qq >= 1 and p == 0:
                            inter = [("out", qq - 1, m) for m in range(2)]
                        atw = []
                        for h in range(2):
                            for kt2 in range(8):
                                if qq == 0 and p == 0 and h == 0 and kt2 >= 2:
                                    atw.append(("vp", kt2))
                                atw.append(("attn", qq, p, h, kt2))
                        # spread `inter` evenly through the attention run
                        out2 = []
                        k = 0
                        step = max(2, (len(atw) - 1) // max(1, len(inter)))
                        for i, w in enumerate(atw):
                            out2.append(w)
                            if inter and k < len(inter) and i % step == step - 1:
                                out2.append(inter[k])
                                k += 1
                        out2.extend(inter[k:])
                        stream.extend(out2)
                stream.append(("outfinal", QQ - 1))

                # ---- emission, software-pipelined (av lags scores) ----
                pend = []

                def flush(n=0):
                    while len(pend) > n:
                        emit_attn_av(*pend.pop(0))

                for w in stream:
                    if w[0] == "kq":
                        emit_kq(w[1], w[2], w[3])
                    elif w[0] == "vp":
                        emit_vp(w[1])
                    elif w[0] == "out":
                        # out-proj reads attn8 written by pending tails
                        flush()
                        flush_tails()
                        emit_out(w[1], w[2])
                    elif w[0] == "outfinal":
                        flush()
                        flush_tails()
                        emit_out_final(w[1])
                    else:
                        rt = emit_attn_scores(*w[1:])
                        pt_t = emit_exp(rt)
                        flush(4)
                        flush_tails(0)
                        pend.append((*w[1:], pt_t))
                flush()
                flush_tails()

    nc.compile()
    return nc


_NC_CACHE = None


def _get_module():
    global _NC_CACHE
    if _NC_CACHE is None:
        _NC_CACHE = _build_module()
    return _NC_CACHE


def _core_inputs(x, y, gnx_w, gnx_b, gny_w, gny_b, qw_q, qb_q, qw_kv, qb_kv,
                 ow, ob):
    f32 = lambda a: np.asarray(a, np.float32)
    x, y = f32(x), f32(y)
    wq, bq = f32(qw_q[0:C]), f32(qb_q[0:C])
    wk, bk = f32(qw_kv[C:2 * C]), f32(qb_kv[C:2 * C])
    wv, bv = f32(qw_kv[2 * C:3 * C]), f32(qb_kv[2 * C:3 * C])
    ow, ob = f32(ow), f32(ob)
    gnx_w, gnx_b = f32(gnx_w), f32(gnx_b)
    gny_w, gny_b = f32(gny_w), f32(gny_b)
    # exact GroupNorm(1,C) stats on the host, folded into weights + biases
    mu_x, rstd_x = x.mean(), 1.0 / np.sqrt(x.var() + EPS)
    mu_y, rstd_y = y.mean(), 1.0 / np.sqrt(y.var() + EPS)
    f8 = lambda w, g, rstd: np.ascontiguousarray(
        (w * (g * rstd)[None, :]).T * WS).astype(E4)
    bq_eff = bq + wq @ (gnx_b - mu_x * rstd_x * gnx_w)
    bk_eff = bk + wk @ (gny_b - mu_y * rstd_y * gny_w)
    bv_eff = bv + wv @ (gny_b - mu_y * rstd_y * gny_w)
    bo_eff = ob + ow @ bv_eff
    # partition-major layout: one contiguous 48B run per partition keeps
    # the vecs DMA at 128 descriptors instead of 1536 7ns-minimum ones
    vecs = np.ascontiguousarray(
        np.stack([bq_eff * SCALE * QS, bk_eff * KS, bo_eff])
        .reshape(3, CO, P).transpose(2, 0, 1)).ravel()
    return {
        "x": np.ascontiguousarray(x).astype(BF16_NP),
        "x8": np.ascontiguousarray(x).astype(E4),
        "y8": np.ascontiguousarray(y).astype(E4),
        "wq8": f8(wq, gnx_w, rstd_x), "wk8": f8(wk, gny_w, rstd_y),
        "wv8": f8(wv, gny_w, rstd_y),
        "wo8": np.ascontiguousarray(ow.T * WS).astype(E4),
        "vecs": vecs,
    }


def kernel(a, b, gn_a_w, gn_a_b, gn_b_w, gn_b_b,
           qkv_a_w, qkv_a_b, qkv_b_w, qkv_b_b,
           out_a_w, out_a_b, out_b_w, out_b_b):
    a = np.asarray(a); b = np.asarray(b)
    nc = _get_module()
    in_maps = []
    for s in range(N):
        # direction a->b : q from a, k/v from b, output -> out_a[s]
        in_maps.append(_core_inputs(a[s], b[s], gn_a_w, gn_a_b, gn_b_w, gn_b_b,
                                    qkv_a_w, qkv_a_b, qkv_b_w, qkv_b_b,
                                    out_a_w, out_a_b))
        # direction b->a : q from b, k/v from a, output -> out_b[s]
        in_maps.append(_core_inputs(b[s], a[s], gn_b_w, gn_b_b, gn_a_w, gn_a_b,
                                    qkv_b_w, qkv_b_b, qkv_a_w, qkv_a_b,
                                    out_b_w, out_b_b))
    res = run_bass_kernel_spmd(nc, in_maps, core_ids=list(range(2 * N)))
    out_a = np.stack([np.asarray(res.results[2 * s]["out"]).view(BF16_NP)
                      if res.results[2 * s]["out"].dtype != BF16_NP
                      else res.results[2 * s]["out"] for s in range(N)])
    out_b = np.stack([np.asarray(res.results[2 * s + 1]["out"]).view(BF16_NP)
                      if res.results[2 * s + 1]["out"].dtype != BF16_NP
                      else res.results[2 * s + 1]["out"] for s in range(N)])
    return out_a.astype(np.float32), out_b.astype(np.float32)
